# revision 1
# baseline (speedup 1.0000x reference)
"""AdaptiveAngleConv Trainium2 kernel.

Computes, for 4 rotated variants of a 3x3 kernel, y[a] = conv2d(x, rot_a(W)) + b
  x: [16, 64, 128, 128] f32, W: [64, 64, 3, 3] f32, b: [64, 1, 1] f32
  out: [4, 16, 64, 128, 128] f32

Strategy: pure data-parallel over batch (2 images per core, 8 cores, no
collectives). Each core runs an implicit-GEMM conv over 4-row output strips
(N=512 = one f32 PSUM bank), with the 4 angle variants merged into the
matmul M dimension as two angle-pairs (M = 2 angles x 64 Cout = 128).

Per strip, 11 matmuls instead of the naive 36 (4 angles x 9 taps at K=64):
 - x is staged in SBUF twice: partitions 0-63 as-is, partitions 64-127
   pre-advanced 2 columns, so taps (kh,0)+(kh,2) fuse into one K=128 matmul
   (3 per angle-pair). Taps (0,1) and (2,1) run as K=64 matmuls.
 - The center tap is rotation-invariant (PERMS[:,4]==4 for all angles), so
   its contribution is computed once per strip (one matmul, [C;C] layout)
   and added during eviction: st = psum + (center + bias) on the VectorE.
   This is provably the minimum matmul count for a dual-shifted-copy layout
   (max vertical/horizontal matching on the 3x3 tap grid minus center is 3).

Weights are pre-permuted on the host into matmul lhsT slot layout
([ap0 slots | shared center | ap1 slots] so the ap0+center half rides the
first, faster wt DMA). Everything TensorEngine-facing is bf16 (rel-err
~2.9e-3 inc. bf16 output staging, well under the 2e-2 gate); accumulation
is f32 in PSUM. Junk warm-up matmuls ramp the PE p-state during the input
DMA head; x loads are chunked and interleaved with compute so image 1
prefetches during image 0; output staging aggregates 2 strips per store DMA
split across HWDGE (sync) and SWDGE (gpsimd) queues.

Cost-model (TimelineSim) predicted exec: ~159-160 us vs a ~150 us
PE-busy floor (704 matmuls x 213 ns) and ~123 us ideal-packing bound.
"""

import numpy as np
import ml_dtypes

import concourse.bass as bass
import concourse.mybir as mybir
from concourse import tile

PERMS = np.array(
    [
        [0, 1, 2, 3, 4, 5, 6, 7, 8],
        [3, 0, 1, 6, 4, 2, 7, 8, 5],
        [6, 3, 0, 7, 4, 1, 8, 5, 2],
        [7, 6, 3, 8, 4, 0, 5, 2, 1],
    ],
    dtype=np.int32,
)

N_CORES = 8
B, CIN, COUT, H, W = 16, 64, 64, 128, 128
BPC = B // N_CORES  # batch images per core
HP, WP = H + 2, W + 2  # padded
STRIP = 4  # output rows per PSUM bank fill
NFREE = STRIP * W  # 512


def _split_multiwait_ctrl(nc):
    """This container's walrus encodes at most one sync-wait per instruction
    (Drain/Matmult/... all hit 'Too many sync wait commands' with >1). Move
    extra waits onto single-wait NOPs preceding the instruction on the same
    engine."""
    nsplit = 0
    for f in nc.m.functions:
        # order multi-waits so the latest-completing sem stays on the real
        # instruction: earlier NoOp waits then retire during its stall
        # window instead of serializing after it. Proxy for completion
        # time: program position of the sem's last updater.
        last_upd = {}
        idx = 0
        for blk in f.blocks:
            for inst in blk.instructions:
                idx += 1
                s2 = inst.sync_info
                if s2 is not None:
                    for u in s2.on_update:
                        last_upd[u.ant_name] = idx
        for blk in f.blocks:
            newlist = []
            for inst in blk.instructions:
                si = inst.sync_info
                if si is not None and len(si.on_wait) > 1:
                    waits = list(si.on_wait)
                    if all(w.wait_mode == "sem-ge-imm" for w in waits):
                        # safe to reorder: >= waits are monotonic
                        waits.sort(key=lambda w: last_upd.get(w.ant_name, 0))
                    for w in waits[:-1]:
                        d = mybir.InstNoOp(
                            name=f"{inst.name}-wsplit{nsplit}", ins=[], outs=[]
                        )
                        nsplit += 1
                        d.engine = inst.engine
                        d.sync_info = mybir.SyncInfo(on_wait=[w], on_update=[])
                        newlist.append(d)
                    si.on_wait = [waits[-1]]
                newlist.append(inst)
            blk.instructions = newlist
    return nsplit


def build_nc(n_batch=BPC, split_ctrl=True, loop_r=None):
    """loop_r: wrap the whole compute in a For_i repeating it loop_r times —
    used only for on-hardware timing (wall-clock delta between two loop_r
    values divided by the iteration delta isolates per-iteration HW time)."""
    bf16 = mybir.dt.bfloat16
    f32 = mybir.dt.float32
    nc = bass.Bass(target_bir_lowering=False)
    xp_d = nc.declare_dram_parameter("xp", [n_batch, CIN, HP, WP], bf16, isOutput=False)
    wt_d = nc.declare_dram_parameter("wt", [128, 11 * 128], bf16, isOutput=False)
    bias_d = nc.declare_dram_parameter("bias2", [128, 1], f32, isOutput=False)
    out_d = nc.declare_dram_parameter(
        "out", [4, n_batch, COUT, H, W], bf16, isOutput=True
    )

    # x loads are chunked by padded-row range so compute can start after the
    # first chunk, and later images' chunks are issued between strips of the
    # previous image (trace order drives scheduler priority).
    CHUNKS = [(0, 10), (10, 30), (30, 55), (55, 80), (80, 105), (105, 130)]
    GROUP = 2  # strips per staged output DMA (8 output rows)

    with tile.TileContext(nc) as tc:
        with (
            tc.tile_pool(name="const", bufs=1) as const_pool,
            tc.tile_pool(name="xpool", bufs=2) as xpool,
            tc.tile_pool(name="psum", bufs=8, space="PSUM") as psum_pool,
            tc.tile_pool(name="stage", bufs=8) as stage_pool,
        ):
            import contextlib

            loop_ctx = tc.For_i(0, loop_r, 1) if loop_r else contextlib.nullcontext()
            with loop_ctx:
                body(nc, const_pool, xpool, psum_pool, stage_pool,
                     xp_d, wt_d, bias_d, out_d, n_batch,
                     use_swdge=loop_r is None)
    if split_ctrl:
        _split_multiwait_ctrl(nc)
    return nc


def body(nc, const_pool, xpool, psum_pool, stage_pool, xp_d, wt_d, bias_d, out_d, n_batch, use_swdge=True):
    gpeng = nc.gpsimd if use_swdge else nc.sync
    bf16 = mybir.dt.bfloat16
    f32 = mybir.dt.float32
    CHUNKS = [(0, 10), (10, 30), (30, 55), (55, 80), (80, 105), (105, 130)]
    GROUP = 2
    if True:
        if True:
            # PE pre-warm: junk matmuls on a zeroed tile ramp the PE p-state
            # while the first x chunk is still in flight. Issued before any
            # DMA so the scheduler gives them the earliest PE priority (a
            # hoisted real Ldweights would head-of-line-block the PE queue
            # on the wt DMA otherwise).
            junk_sb = const_pool.tile([128, 256], bf16)
            nc.vector.memset(junk_sb[:], 0)
            for w in range(14):
                jps = psum_pool.tile([128, 256], f32, tag="ps", name=f"jps{w}")
                nc.tensor.matmul(jps[:], junk_sb[:, 0:128], junk_sb[:, 0:256])

            # wt split across HWDGE+SWDGE so neither device serializes it
            # behind the x-chunk loads
            wt_sb = const_pool.tile([128, 11 * 128], bf16)
            nc.sync.dma_start(wt_sb[:, 0:768], wt_d[:, 0:768])
            gpeng.dma_start(wt_sb[:, 768:1408], wt_d[:, 768:1408])
            bias_sb = const_pool.tile([128, 1], f32)

            xtiles = [
                xpool.tile([128, HP * WP], bf16, tag="xt", name=f"xt{b}")
                for b in range(n_batch)
            ]

            chunk_i = [0]

            def load_chunk(b, lo, hi):
                # alternate chunks between the HWDGE (sync) and SWDGE (gpsimd)
                # devices so consecutive chunk loads proceed in parallel
                eng = nc.sync if chunk_i[0] % 2 == 0 else gpeng
                chunk_i[0] += 1
                xt = xtiles[b]
                flat = xp_d[b].rearrange("c h w -> c (h w)")
                # lower copy: padded rows [lo, hi)
                eng.dma_start(xt[0:64, lo * WP : hi * WP], flat[:, lo * WP : hi * WP])
                # upper copy pre-advanced 2 columns: upper[f] = flat[f+2]
                d0 = max(lo * WP - 2, 0)
                eng.dma_start(
                    xt[64:128, d0 : hi * WP - 2], flat[:, d0 + 2 : hi * WP]
                )

            # pending chunk loads, issued interleaved with strips
            pending = [(b, lo, hi) for b in range(n_batch) for (lo, hi) in CHUNKS]
            # image 0 chunk 0 must come first, then bias (needed only at evict)
            load_chunk(*pending.pop(0))
            gpeng.dma_start(bias_sb[:], bias_d[:])

            n_groups = H // (STRIP * GROUP)
            for b in range(n_batch):
                xt = xtiles[b]
                xv = xt[:].rearrange("p (h w) -> p h w", w=WP)

                for g in range(n_groups):
                    rg = g * GROUP * STRIP  # first output row of the group
                    # shared center-tap contribution per strip (angle-invariant):
                    # c2sb = [C; C] + bias, reused by both angle-pair evictions
                    c2sbs = []
                    for si in range(GROUP):
                        s_i = g * GROUP + si
                        r0 = s_i * STRIP
                        # required: image-b chunks covering rows <= r0+5
                        while pending and pending[0][0] == b and pending[0][1] <= r0 + 5:
                            load_chunk(*pending.pop(0))
                        # prefetch: one lookahead chunk every 4th strip
                        if pending and s_i % 4 == 2:
                            load_chunk(*pending.pop(0))
                        cps = psum_pool.tile([128, NFREE], f32, tag="ps", name=f"cps{s_i}")
                        nc.tensor.matmul(
                            cps[:],
                            wt_sb[0:64, 5 * 128 : 6 * 128],
                            xv[0:64, r0 + 1 : r0 + 1 + STRIP, 1 : 1 + W],
                        )
                        c2sb = stage_pool.tile(
                            [128, NFREE], f32, tag="c2", name=f"c2_{s_i}"
                        )
                        nc.scalar.activation(
                            c2sb[:],
                            cps[:],
                            mybir.ActivationFunctionType.Identity,
                            bias=bias_sb[:],
                        )
                        c2sbs.append(c2sb)
                    for ap in range(2):
                        st = stage_pool.tile(
                            [128, GROUP * NFREE], bf16, tag="st", name=f"st{b}_{g}_{ap}"
                        )
                        for si in range(GROUP):
                            s_i = g * GROUP + si
                            r0 = s_i * STRIP
                            ps = psum_pool.tile([128, NFREE], f32, tag="ps")
                            base = 0 if ap == 0 else 6
                            # K=128 pairs: taps (kh,0) lower + (kh,2) upper
                            for j in range(3):
                                s = base + j
                                nc.tensor.matmul(
                                    ps[:],
                                    wt_sb[:, s * 128 : (s + 1) * 128],
                                    xv[:, r0 + j : r0 + j + STRIP, 0:W],
                                    start=(j == 0),
                                    stop=False,
                                )
                            # K=64 singles: taps (0,1) and (2,1) from lower copy
                            nc.tensor.matmul(
                                ps[:],
                                wt_sb[0:64, (base + 3) * 128 : (base + 4) * 128],
                                xv[0:64, r0 : r0 + STRIP, 1 : 1 + W],
                                start=False,
                                stop=False,
                            )
                            nc.tensor.matmul(
                                ps[:],
                                wt_sb[0:64, (base + 4) * 128 : (base + 5) * 128],
                                xv[0:64, r0 + 2 : r0 + 2 + STRIP, 1 : 1 + W],
                                start=False,
                                stop=True,
                            )
                            # eviction: st = ps + (center + bias), DVE only
                            st_slice = st[:, si * NFREE : (si + 1) * NFREE]
                            nc.vector.tensor_add(st_slice, ps[:], c2sbs[si][:])
                        last_group = b == n_batch - 1 and g == n_groups - 1
                        if not last_group:
                            for al in range(2):
                                a = 2 * ap + al
                                eng = nc.sync if al == 0 else gpeng
                                eng.dma_start(
                                    out_d[a, b, :, rg : rg + GROUP * STRIP, :],
                                    st[al * 64 : (al + 1) * 64, :],
                                )
                        else:
                            # final group: per-strip stores so earlier strips'
                            # transfers overlap the last strips' matmuls; the
                            # very last strip uses ONE fused two-angle store
                            # (single issue chain ends earlier than two
                            # staggered transfers)
                            for si in range(GROUP):
                                r0 = rg + si * STRIP
                                if si == GROUP - 1:
                                    nc.sync.dma_start(
                                        out_d[2 * ap : 2 * ap + 2, b, :, r0 : r0 + STRIP, :],
                                        st[:, si * NFREE : (si + 1) * NFREE],
                                    )
                                    continue
                                for al in range(2):
                                    a = 2 * ap + al
                                    eng = nc.sync if al == 0 else gpeng
                                    eng.dma_start(
                                        out_d[a, b, :, r0 : r0 + STRIP, :],
                                        st[
                                            al * 64 : (al + 1) * 64,
                                            si * NFREE : (si + 1) * NFREE,
                                        ],
                                    )


def prep_weights(weight, bias):
    """wt: [128, 11*128] bf16 lhsT layout; bias2: [128, 1] f32.

    Slots 0-4: angle-pair 0, slot 5: shared center, slots 6-10: angle-pair 1
    (center rides the first wt DMA half together with ap0). Per ap: slots
    +0..+2 are K=128 pairs {tap (kh,0) lower | tap (kh,2) upper}; +3 = single
    tap (0,1); +4 = single tap (2,1). The center tap (flat 4) is
    rotation-invariant (PERMS[:,4]==4), computed once per strip.
    """
    wflat = np.asarray(weight, np.float32).reshape(COUT, CIN, 9)
    # L[t][c, a, o] = wflat[o, c, PERMS[a, t]]
    L = wflat[:, :, PERMS].transpose(3, 1, 2, 0)  # [9, c, a, o]
    wt = np.zeros((128, 11, 128), np.float32)
    for ap in range(2):
        base = 0 if ap == 0 else 6
        La = L[:, :, 2 * ap : 2 * ap + 2, :].reshape(9, CIN, 128)  # [t, c, m]
        for j in range(3):
            wt[0:64, base + j] = La[3 * j + 0]  # tap (j, 0) lower
            wt[64:128, base + j] = La[3 * j + 2]  # tap (j, 2) upper
        wt[0:64, base + 3] = La[1]  # tap (0, 1)
        wt[0:64, base + 4] = La[7]  # tap (2, 1)
    # shared center at slot 5 (so it rides the fast HWDGE wt half):
    # lhsT[c, al*64+o] = W[o, c, 4] duplicated for both angles
    w4 = wflat[:, :, 4].T  # [c, o]
    wt[0:64, 5] = np.concatenate([w4, w4], axis=1)
    wt = wt.reshape(128, 11 * 128).astype(ml_dtypes.bfloat16)
    bias2 = np.tile(np.asarray(bias, np.float32).reshape(COUT), 2)[:, None]
    return wt, np.ascontiguousarray(bias2, np.float32)


def prep_x(x):
    """Pad to 130x130 and convert to bf16. [B, CIN, HP, WP] bf16."""
    xp = np.zeros((x.shape[0], CIN, HP, WP), np.float32)
    xp[:, :, 1 : H + 1, 1 : W + 1] = np.asarray(x, np.float32)
    return xp.astype(ml_dtypes.bfloat16)


_CACHE = {}


def _enable_persistent_compile_cache():
    # NEFF compiles take 1-7 minutes; jax's persistent cache serializes the
    # compiled executable (NEFF included) so fresh processes skip the
    # recompile. Best-effort: ignored if the PJRT backend can't serialize.
    try:
        import jax

        jax.config.update("jax_compilation_cache_dir", "/tmp/jax_comp_cache")
        jax.config.update("jax_persistent_cache_min_compile_time_secs", 1.0)
    except Exception:
        pass


def kernel(x, weight, bias):
    from concourse import bass2jax as b2j

    _enable_persistent_compile_cache()

    x = np.asarray(x)
    in_dtype = x.dtype
    xp = prep_x(x)  # [B, CIN, HP, WP] bf16
    wt, bias2 = prep_weights(weight, bias)

    if "nc" not in _CACHE:
        _CACHE["nc"] = build_nc()
    nc = _CACHE["nc"]
    in_maps = [
        {"xp": xp[i * BPC : (i + 1) * BPC], "wt": wt, "bias2": bias2}
        for i in range(N_CORES)
    ]
    results = b2j.run_bass_via_pjrt(nc, in_maps, n_cores=N_CORES)
    out = np.stack([r["out"] for r in results])  # [N_CORES, 4, BPC, ...]
    out = out.transpose(1, 0, 2, 3, 4, 5).reshape(4, B, COUT, H, W)
    return out.astype(in_dtype)



# revision 8
# speedup vs baseline: 1.1917x; 1.1917x over previous
"""AdaptiveAngleConv Trainium2 kernel.

Computes, for 4 rotated variants of a 3x3 kernel, y[a] = conv2d(x, rot_a(W)) + b
  x: [16, 64, 128, 128] f32, W: [64, 64, 3, 3] f32, b: [64, 1, 1] f32
  out: [4, 16, 64, 128, 128] f32

Strategy: pure data-parallel over batch (2 images per core, 8 cores, no
collectives). Each core runs an implicit-GEMM conv over 4-row output strips
(N=512 = one f32 PSUM bank), with the 4 angle variants merged into the
matmul M dimension as two angle-pairs (M = 2 angles x 64 Cout = 128).

Per strip, 9 matmuls instead of the naive 36 (4 angles x 9 taps at K=64) —
the ideal-packing minimum for this tap structure:
 - x is staged in SBUF as TWO dual-copy regions: region 1 holds x as-is in
   partitions 0-63 and x pre-advanced 2 columns in 64-127, fusing taps
   (kh,0)+(kh,2) into one K=128 matmul (3 per angle-pair). Region 2 holds
   x as-is in 0-63 and x pre-advanced 2 ROWS in 64-127, fusing the
   remaining middle-column taps (0,1)+(2,1) into one more K=128 matmul per
   angle-pair (4 accumulating matmuls per chain, no K=64 stragglers).
 - The center tap is rotation-invariant (PERMS[:,4]==4 for all angles), so
   its contribution is computed once per strip (one matmul, [C;C] layout)
   and added during eviction: st = psum + (center + bias) on the VectorE.

Weights are pre-permuted on the host into matmul lhsT slot layout
([ap0 slots | shared center | ap1 slots] so the ap0+center half rides the
first, faster wt DMA). Everything TensorEngine-facing is bf16 (rel-err
~2.9e-3 inc. bf16 output staging, well under the 2e-2 gate); accumulation
is f32 in PSUM. Junk warm-up matmuls ramp the PE p-state during the input
DMA head; x loads are chunked and interleaved with compute so image 1
prefetches during image 0; output staging aggregates 2 strips per store DMA
split across HWDGE (sync) and SWDGE (gpsimd) queues.

Cost model: matmul time is output-free-size x pe_cycle regardless of K, so
the 576 matmuls put the PE floor at ~123 us (576 x 213 ns); input DMA is
~17.3 MB (4 x copies) + 16.8 MB out = ~96 us, still under the PE floor.
"""

import numpy as np
import ml_dtypes

import concourse.bass as bass
import concourse.mybir as mybir
from concourse import tile

PERMS = np.array(
    [
        [0, 1, 2, 3, 4, 5, 6, 7, 8],
        [3, 0, 1, 6, 4, 2, 7, 8, 5],
        [6, 3, 0, 7, 4, 1, 8, 5, 2],
        [7, 6, 3, 8, 4, 0, 5, 2, 1],
    ],
    dtype=np.int32,
)

N_CORES = 8
B, CIN, COUT, H, W = 16, 64, 64, 128, 128
BPC = B // N_CORES  # batch images per core
HP, WP = H + 2, W + 2  # padded
STRIP = 4  # output rows per PSUM bank fill
NFREE = STRIP * W  # 512


def _split_multiwait_ctrl(nc):
    """This container's walrus encodes at most one sync-wait per instruction
    (Drain/Matmult/... all hit 'Too many sync wait commands' with >1). Move
    extra waits onto single-wait NOPs preceding the instruction on the same
    engine."""
    nsplit = 0
    for f in nc.m.functions:
        # order multi-waits so the latest-completing sem stays on the real
        # instruction: earlier NoOp waits then retire during its stall
        # window instead of serializing after it. Proxy for completion
        # time: program position of the sem's last updater.
        last_upd = {}
        idx = 0
        for blk in f.blocks:
            for inst in blk.instructions:
                idx += 1
                s2 = inst.sync_info
                if s2 is not None:
                    for u in s2.on_update:
                        last_upd[u.ant_name] = idx
        for blk in f.blocks:
            newlist = []
            for inst in blk.instructions:
                si = inst.sync_info
                if si is not None and len(si.on_wait) > 1:
                    waits = list(si.on_wait)
                    if all(w.wait_mode == "sem-ge-imm" for w in waits):
                        # safe to reorder: >= waits are monotonic
                        waits.sort(key=lambda w: last_upd.get(w.ant_name, 0))
                    for w in waits[:-1]:
                        d = mybir.InstNoOp(
                            name=f"{inst.name}-wsplit{nsplit}", ins=[], outs=[]
                        )
                        nsplit += 1
                        d.engine = inst.engine
                        d.sync_info = mybir.SyncInfo(on_wait=[w], on_update=[])
                        newlist.append(d)
                    si.on_wait = [waits[-1]]
                newlist.append(inst)
            blk.instructions = newlist
    return nsplit


def build_nc(n_batch=BPC, split_ctrl=True, loop_r=None):
    """loop_r: wrap the whole compute in a For_i repeating it loop_r times —
    used only for on-hardware timing (wall-clock delta between two loop_r
    values divided by the iteration delta isolates per-iteration HW time)."""
    bf16 = mybir.dt.bfloat16
    f32 = mybir.dt.float32
    nc = bass.Bass(target_bir_lowering=False)
    xp_d = nc.declare_dram_parameter("xp", [n_batch, CIN, HP, WP], bf16, isOutput=False)
    wt_d = nc.declare_dram_parameter("wt", [128, 9 * 128], bf16, isOutput=False)
    bias_d = nc.declare_dram_parameter("bias2", [128, 1], f32, isOutput=False)
    out_d = nc.declare_dram_parameter(
        "out", [4, n_batch, COUT, H, W], bf16, isOutput=True
    )

    # x loads are chunked by padded-row range so compute can start after the
    # first chunk, and later images' chunks are issued between strips of the
    # previous image (trace order drives scheduler priority).
    CHUNKS = [(0, 10), (10, 30), (30, 55), (55, 80), (80, 105), (105, 130)]
    GROUP = 2  # strips per staged output DMA (8 output rows)

    with tile.TileContext(nc) as tc:
        with (
            tc.tile_pool(name="const", bufs=1) as const_pool,
            tc.tile_pool(name="xpool", bufs=2) as xpool,
            tc.tile_pool(name="psum", bufs=8, space="PSUM") as psum_pool,
            tc.tile_pool(name="stage", bufs=8) as stage_pool,
        ):
            import contextlib

            loop_ctx = tc.For_i(0, loop_r, 1) if loop_r else contextlib.nullcontext()
            with loop_ctx:
                body(nc, const_pool, xpool, psum_pool, stage_pool,
                     xp_d, wt_d, bias_d, out_d, n_batch,
                     use_swdge=loop_r is None)
    if split_ctrl:
        _split_multiwait_ctrl(nc)
    return nc


def body(nc, const_pool, xpool, psum_pool, stage_pool, xp_d, wt_d, bias_d, out_d, n_batch, use_swdge=True):
    gpeng = nc.gpsimd if use_swdge else nc.sync
    bf16 = mybir.dt.bfloat16
    f32 = mybir.dt.float32
    CHUNKS = [(0, 10), (10, 30), (30, 55), (55, 80), (80, 105), (105, 130)]
    GROUP = 2
    if True:
        if True:
            # PE pre-warm: junk matmuls on a zeroed tile ramp the PE p-state
            # while the first x chunk is still in flight. Issued before any
            # DMA so the scheduler gives them the earliest PE priority (a
            # hoisted real Ldweights would head-of-line-block the PE queue
            # on the wt DMA otherwise).
            junk_sb = const_pool.tile([128, 256], bf16)
            nc.vector.memset(junk_sb[:], 0)
            for w in range(14):
                jps = psum_pool.tile([128, 256], f32, tag="ps", name=f"jps{w}")
                nc.tensor.matmul(jps[:], junk_sb[:, 0:128], junk_sb[:, 0:256])

            # wt split across HWDGE+SWDGE so neither device serializes it
            # behind the x-chunk loads
            wt_sb = const_pool.tile([128, 9 * 128], bf16)
            nc.sync.dma_start(wt_sb[:, 0:640], wt_d[:, 0:640])
            gpeng.dma_start(wt_sb[:, 640:1152], wt_d[:, 640:1152])
            bias_sb = const_pool.tile([128, 1], f32)

            R2 = HP * WP  # column offset of region 2 within an x tile
            xtiles = [
                xpool.tile([128, 2 * HP * WP], bf16, tag="xt", name=f"xt{b}")
                for b in range(n_batch)
            ]

            chunk_i = [0]

            def load_chunk(b, lo, hi):
                # alternate chunks between the HWDGE (sync) and SWDGE (gpsimd)
                # devices so consecutive chunk loads proceed in parallel;
                # region 2 rides the opposite device from region 1
                eng = nc.sync if chunk_i[0] % 2 == 0 else gpeng
                eng2 = gpeng if chunk_i[0] % 2 == 0 else nc.sync
                chunk_i[0] += 1
                xt = xtiles[b]
                flat = xp_d[b].rearrange("c h w -> c (h w)")
                # region 1 lower copy: padded rows [lo, hi)
                eng.dma_start(xt[0:64, lo * WP : hi * WP], flat[:, lo * WP : hi * WP])
                # region 1 upper copy pre-advanced 2 columns: upper[f] = flat[f+2]
                d0 = max(lo * WP - 2, 0)
                eng.dma_start(
                    xt[64:128, d0 : hi * WP - 2], flat[:, d0 + 2 : hi * WP]
                )
                # region 2 lower copy: x again
                eng2.dma_start(
                    xt[0:64, R2 + lo * WP : R2 + hi * WP], flat[:, lo * WP : hi * WP]
                )
                # region 2 upper copy pre-advanced 2 rows: upper[f] = flat[f+2*WP]
                d2 = max(lo * WP - 2 * WP, 0)
                eng2.dma_start(
                    xt[64:128, R2 + d2 : R2 + hi * WP - 2 * WP],
                    flat[:, d2 + 2 * WP : hi * WP],
                )

            # pending chunk loads, issued interleaved with strips
            pending = [(b, lo, hi) for b in range(n_batch) for (lo, hi) in CHUNKS]
            # image 0 chunk 0 must come first, then bias (needed only at evict)
            load_chunk(*pending.pop(0))
            gpeng.dma_start(bias_sb[:], bias_d[:])

            n_groups = H // (STRIP * GROUP)
            for b in range(n_batch):
                xt = xtiles[b]
                xv = xt[:, 0 : HP * WP].rearrange("p (h w) -> p h w", w=WP)
                xv2 = xt[:, R2 : 2 * R2].rearrange("p (h w) -> p h w", w=WP)

                for g in range(n_groups):
                    rg = g * GROUP * STRIP  # first output row of the group
                    # shared center-tap contribution per strip (angle-invariant):
                    # c2sb = [C; C] + bias, reused by both angle-pair evictions
                    c2sbs = []
                    for si in range(GROUP):
                        s_i = g * GROUP + si
                        r0 = s_i * STRIP
                        # required: image-b chunks covering rows <= r0+5
                        while pending and pending[0][0] == b and pending[0][1] <= r0 + 5:
                            load_chunk(*pending.pop(0))
                        # prefetch: one lookahead chunk every 4th strip
                        if pending and s_i % 4 == 2:
                            load_chunk(*pending.pop(0))
                        cps = psum_pool.tile([128, NFREE], f32, tag="ps", name=f"cps{s_i}")
                        nc.tensor.matmul(
                            cps[:],
                            wt_sb[0:64, 4 * 128 : 5 * 128],
                            xv[0:64, r0 + 1 : r0 + 1 + STRIP, 1 : 1 + W],
                        )
                        c2sb = stage_pool.tile(
                            [128, NFREE], f32, tag="c2", name=f"c2_{s_i}"
                        )
                        nc.scalar.activation(
                            c2sb[:],
                            cps[:],
                            mybir.ActivationFunctionType.Identity,
                            bias=bias_sb[:],
                        )
                        c2sbs.append(c2sb)
                    for ap in range(2):
                        st = stage_pool.tile(
                            [128, GROUP * NFREE], bf16, tag="st", name=f"st{b}_{g}_{ap}"
                        )
                        for si in range(GROUP):
                            s_i = g * GROUP + si
                            r0 = s_i * STRIP
                            ps = psum_pool.tile([128, NFREE], f32, tag="ps")
                            base = 0 if ap == 0 else 5
                            # K=128 pairs: taps (kh,0) lower + (kh,2) upper
                            for j in range(3):
                                s = base + j
                                nc.tensor.matmul(
                                    ps[:],
                                    wt_sb[:, s * 128 : (s + 1) * 128],
                                    xv[:, r0 + j : r0 + j + STRIP, 0:W],
                                    start=(j == 0),
                                    stop=False,
                                )
                            # K=128 vertical pair: taps (0,1) lower + (2,1)
                            # upper (region 2's copy is pre-advanced 2 rows)
                            nc.tensor.matmul(
                                ps[:],
                                wt_sb[:, (base + 3) * 128 : (base + 4) * 128],
                                xv2[:, r0 : r0 + STRIP, 1 : 1 + W],
                                start=False,
                                stop=True,
                            )
                            # eviction: st = ps + (center + bias), DVE only
                            st_slice = st[:, si * NFREE : (si + 1) * NFREE]
                            nc.vector.tensor_add(st_slice, ps[:], c2sbs[si][:])
                        last_group = b == n_batch - 1 and g == n_groups - 1
                        if not last_group:
                            for al in range(2):
                                a = 2 * ap + al
                                eng = nc.sync if al == 0 else gpeng
                                eng.dma_start(
                                    out_d[a, b, :, rg : rg + GROUP * STRIP, :],
                                    st[al * 64 : (al + 1) * 64, :],
                                )
                        else:
                            # final group: per-strip stores so earlier strips'
                            # transfers overlap the last strips' matmuls; the
                            # very last strip uses ONE fused two-angle store
                            # (single issue chain ends earlier than two
                            # staggered transfers)
                            for si in range(GROUP):
                                r0 = rg + si * STRIP
                                if si == GROUP - 1:
                                    nc.sync.dma_start(
                                        out_d[2 * ap : 2 * ap + 2, b, :, r0 : r0 + STRIP, :],
                                        st[:, si * NFREE : (si + 1) * NFREE],
                                    )
                                    continue
                                for al in range(2):
                                    a = 2 * ap + al
                                    eng = nc.sync if al == 0 else gpeng
                                    eng.dma_start(
                                        out_d[a, b, :, r0 : r0 + STRIP, :],
                                        st[
                                            al * 64 : (al + 1) * 64,
                                            si * NFREE : (si + 1) * NFREE,
                                        ],
                                    )


def prep_weights(weight, bias):
    """wt: [128, 9*128] bf16 lhsT layout; bias2: [128, 1] f32.

    Slots 0-3: angle-pair 0, slot 4: shared center, slots 5-8: angle-pair 1
    (center rides the first wt DMA half together with ap0). Per ap: slots
    +0..+2 are K=128 pairs {tap (kh,0) lower | tap (kh,2) upper} against
    region 1; +3 is the K=128 pair {tap (0,1) lower | tap (2,1) upper}
    against region 2 (upper copy pre-advanced 2 rows). The center tap
    (flat 4) is rotation-invariant (PERMS[:,4]==4), computed once per strip.
    """
    wflat = np.asarray(weight, np.float32).reshape(COUT, CIN, 9)
    # L[t][c, a, o] = wflat[o, c, PERMS[a, t]]
    L = wflat[:, :, PERMS].transpose(3, 1, 2, 0)  # [9, c, a, o]
    wt = np.zeros((128, 9, 128), np.float32)
    for ap in range(2):
        base = 0 if ap == 0 else 5
        La = L[:, :, 2 * ap : 2 * ap + 2, :].reshape(9, CIN, 128)  # [t, c, m]
        for j in range(3):
            wt[0:64, base + j] = La[3 * j + 0]  # tap (j, 0) lower
            wt[64:128, base + j] = La[3 * j + 2]  # tap (j, 2) upper
        wt[0:64, base + 3] = La[1]  # tap (0, 1) lower
        wt[64:128, base + 3] = La[7]  # tap (2, 1) upper (2-rows-advanced)
    # shared center at slot 4 (so it rides the fast HWDGE wt half):
    # lhsT[c, al*64+o] = W[o, c, 4] duplicated for both angles
    w4 = wflat[:, :, 4].T  # [c, o]
    wt[0:64, 4] = np.concatenate([w4, w4], axis=1)
    wt = wt.reshape(128, 9 * 128).astype(ml_dtypes.bfloat16)
    bias2 = np.tile(np.asarray(bias, np.float32).reshape(COUT), 2)[:, None]
    return wt, np.ascontiguousarray(bias2, np.float32)


def prep_x(x):
    """Pad to 130x130 and convert to bf16. [B, CIN, HP, WP] bf16."""
    xp = np.zeros((x.shape[0], CIN, HP, WP), np.float32)
    xp[:, :, 1 : H + 1, 1 : W + 1] = np.asarray(x, np.float32)
    return xp.astype(ml_dtypes.bfloat16)


_CACHE = {}


def _enable_persistent_compile_cache():
    # NEFF compiles take 1-7 minutes; jax's persistent cache serializes the
    # compiled executable (NEFF included) so fresh processes skip the
    # recompile. Best-effort: ignored if the PJRT backend can't serialize.
    try:
        import jax

        jax.config.update("jax_compilation_cache_dir", "/tmp/jax_comp_cache")
        jax.config.update("jax_persistent_cache_min_compile_time_secs", 1.0)
    except Exception:
        pass


def kernel(x, weight, bias):
    from concourse import bass2jax as b2j

    _enable_persistent_compile_cache()

    x = np.asarray(x)
    in_dtype = x.dtype
    xp = prep_x(x)  # [B, CIN, HP, WP] bf16
    wt, bias2 = prep_weights(weight, bias)

    if "nc" not in _CACHE:
        _CACHE["nc"] = build_nc()
    nc = _CACHE["nc"]
    in_maps = [
        {"xp": xp[i * BPC : (i + 1) * BPC], "wt": wt, "bias2": bias2}
        for i in range(N_CORES)
    ]
    results = b2j.run_bass_via_pjrt(nc, in_maps, n_cores=N_CORES)
    out = np.stack([r["out"] for r in results])  # [N_CORES, 4, BPC, ...]
    out = out.transpose(1, 0, 2, 3, 4, 5).reshape(4, B, COUT, H, W)
    return out.astype(in_dtype)

